# revision 14
# baseline (speedup 1.0000x reference)
"""Trainium2 Bass kernel for the continuous-convolution (CConv) GNN layer.

Math (per output point n, P=32 neighbors, 4x4 bilinear kernel grid, 64->64 ch):
    gathered = features[receivers]                      # [N,P,64]
    win      = relu(1 - |r|^2/ws^2)^a                   # radial window
    gy,gx    = clip((r/ws + 1)*1.5, 0, 3)               # grid coords
    bilinear -> tent weights  w_j = relu(1 - |g - j|)   # j = 0..3 (exact)
    M[n,g]   = sum_p win * wy[jy] * wx[jx] * gathered   # g = 4*jy+jx
    out[n]   = (sum_g M[n,g] @ K[g]) / P + bias

Device mapping (8 NeuronCores, data-parallel over points):
  * 6272 points/core (50176 padded), edges blocked 128 = 4 points x 32 nbrs.
  * Gather: Q7 dma_gather from the HBM feature table. int16 indices cover
    all 50000 rows by pointing the source AP at row 25000 (signed offsets
    reach both halves); a trailing pad block per call keeps the last index
    non-negative (the ucode trims trailing negatives).
  * Stage 1 (PE): per 128-edge block  Mt = G^T @ U : lhsT = gathered G
    [128e, 64ch], rhs = U [128e, 64] block-diagonal bilinear weights
    (4 points x 16 bins) -> psum [64ch, 4pt*16g].
  * Stage 2 (PE): out^T[oc, pts] += K_g^T[oc,ch] @ Mt_g[ch, pts], 16 bins
    accumulated in PSUM; then *1/P + bias on DVE; out stored transposed,
    host transposes back (pure layout).
"""

import sys

sys.path.insert(0, "/opt/trn_rl_repo")

import dataclasses
from contextlib import ExitStack

import numpy as np

N_FULL = 50000
HALF = 25000             # gather base row: int16 idx = r - HALF
P_NBR = 32
CIN = 64
COUT = 64
G_BINS = 16
NCORES = 8
NPTS = 6272              # padded points per core; 8*6272 = 50176 >= 50000
NBLK = NPTS // 4         # 1568 blocks of 128 edges
C_BLK = 56               # real blocks per pipeline chunk
NCHUNK = NBLK // C_BLK   # 28
GCALL = 8                # blocks per dma_gather call (1024-descriptor ring cap)
NCALL_C = C_BLK // GCALL  # gather calls per chunk (7)
PERCALL = GCALL * 128    # indices per gather call (1024)
PC_COLS = PERCALL // 16  # idx columns per call (64)
PTS_CHUNK = C_BLK * 4    # 224 points produced per chunk

_prog_cache = {}
LAST_EXEC_NS = None


def _build_nc(a_exp, inv_ws2, s15):
    import concourse.bacc as bacc
    import concourse.bass as bass
    import concourse.mybir as mybir
    from concourse.tile import TileContext
    from concourse.vector_clock import ScopedClock, VectorClock

    f32 = mybir.dt.float32
    f16 = mybir.dt.float16
    i16 = mybir.dt.int16
    Alu = mybir.AluOpType
    Act = mybir.ActivationFunctionType

    class TC(TileContext):
        # The stock final drain packs every outstanding semaphore wait onto a
        # single Drain instruction; walrus here accepts at most one sync-wait
        # per CTRL instruction. Emit one drain per outstanding sem lane.
        def _drain_and_barrier(self, tick_clock, wait_clock):
            nc = self.nc
            ticks = eval(repr(tick_clock.global_clock).replace("VectorClock", ""))
            nz = [i for i, t in enumerate(ticks) if t > 0]
            if not nz:
                nc.sync.drain()
            for i in nz:
                part = [ticks[j] if j == i else 0 for j in range(len(ticks))]
                d = nc.sync.drain()
                wait_clock.add_sem_waits(d.ins, ScopedClock({None: VectorClock(part)}))
            nc.all_engine_barrier()
            popped = nc._tile_sem_poison_stack.pop()
            assert popped is self._sem_poison
            nc.clear_and_free_semaphores(list(self.sems.allocated().values()))
            nc.all_engine_barrier()

    def bc(view, dims, extra_off=0):
        # hand-built access pattern: keep partition dim, replace free dims
        return dataclasses.replace(
            view,
            ap=[view.ap[0]] + [list(d) for d in dims],
            offset=view.offset + extra_off,
        )

    nc = bacc.Bacc(
        "TRN2", target_bir_lowering=False, debug=False, num_swdge_queues=4
    )
    feat = nc.declare_dram_parameter("feat", [N_FULL, 2 * CIN], f16, isOutput=False)
    idxs = nc.declare_dram_parameter("idxs", [128, NBLK * 8], i16, isOutput=False)
    posy = nc.declare_dram_parameter("posy", [128, NBLK], f32, isOutput=False)
    posx = nc.declare_dram_parameter("posx", [128, NBLK], f32, isOutput=False)
    kmat = nc.declare_dram_parameter("kmat", [CIN, G_BINS * COUT], f16, isOutput=False)
    bias = nc.declare_dram_parameter("bias", [COUT, 1], f32, isOutput=False)
    iot4 = nc.declare_dram_parameter("iot4", [128, 4], f32, isOutput=False)
    c15d = nc.declare_dram_parameter("c15d", [128, 1], f32, isOutput=False)
    outT = nc.declare_dram_parameter("outT", [COUT, NPTS], f32, isOutput=True)

    with TC(nc) as tc, ExitStack() as ctx:
        const = ctx.enter_context(tc.tile_pool(name="const", bufs=1))
        gpool = ctx.enter_context(tc.tile_pool(name="g", bufs=3))
        wpool = ctx.enter_context(tc.tile_pool(name="w", bufs=3))
        mpool = ctx.enter_context(tc.tile_pool(name="mt", bufs=3))
        opool = ctx.enter_context(tc.tile_pool(name="ot", bufs=3))
        pspool = ctx.enter_context(tc.tile_pool(name="ps", bufs=3, space="PSUM"))

        idx_sb = const.tile([128, NBLK * 8], i16)
        posy_sb = const.tile([128, NBLK], f32)
        posx_sb = const.tile([128, NBLK], f32)
        kmat_sb = const.tile([CIN, G_BINS * COUT], f16)
        bias_sb = const.tile([COUT, 1], f32)
        iota4 = const.tile([128, 4], f32)
        c15 = const.tile([128, 1], f32)
        # U tiles keep their block-diagonal zero regions across chunks
        u_bufs = [
            const.tile([128, C_BLK * 64], f16, tag="u0", name="u0"),
            const.tile([128, C_BLK * 64], f16, tag="u1", name="u1"),
            const.tile([128, C_BLK * 64], f16, tag="u2", name="u2"),
        ]

        nc.sync.dma_start(out=idx_sb[:], in_=idxs[:])
        nc.sync.dma_start(out=posy_sb[:], in_=posy[:])
        nc.sync.dma_start(out=posx_sb[:], in_=posx[:])
        nc.sync.dma_start(out=kmat_sb[:], in_=kmat[:])
        nc.sync.dma_start(out=bias_sb[:], in_=bias[:])
        nc.sync.dma_start(out=iota4[:], in_=iot4[:])
        nc.sync.dma_start(out=c15[:], in_=c15d[:])
        nc.vector.memset(u_bufs[0][:], 0.0)
        nc.vector.memset(u_bufs[1][:], 0.0)
        nc.vector.memset(u_bufs[2][:], 0.0)

        import os as _os

        _nchunk = int(_os.environ.get("KERNEL_NCHUNK", NCHUNK))
        _dbg = _os.environ.get("KERNEL_DEBUG", "full")
        for ci in range(_nchunk):
            c0 = ci * C_BLK
            u = u_bufs[ci % 3]

            # ---- gather: 56 blocks of feature rows, 8 blocks per call ----
            # (fp16 rows padded to 128ch = 256B elements; desc-gen spread
            #  over the 4 SWDGE queues = 4 Q7 core pairs)
            gt = gpool.tile([128, C_BLK * 2 * CIN], f16, tag="gt")
            for sc in range(NCALL_C):
                gv = dataclasses.replace(
                    gt[:],
                    ap=[gt[:].ap[0], [2 * CIN, GCALL], [1, 2 * CIN]],
                    offset=gt[:].offset + sc * GCALL * 2 * CIN,
                )
                col0 = (ci * NCALL_C + sc) * PC_COLS
                nc.gpsimd.dma_gather(
                    out_ap=gv,
                    in_ap=feat[HALF:, :],
                    idxs_ap=idx_sb[:, col0 : col0 + PC_COLS],
                    num_idxs=PERCALL,
                    num_idxs_reg=PERCALL,
                    elem_size=2 * CIN,
                    elem_step=2 * CIN,
                    queue_num=(ci * NCALL_C + sc) % 4,
                )

            if _dbg == "gather":
                ot = opool.tile([COUT, PTS_CHUNK], f32, tag="ot")
                nc.vector.tensor_copy(ot[:], gt[0:COUT, 0:PTS_CHUNK])
                nc.sync.dma_start(
                    out=outT[:, ci * PTS_CHUNK : (ci + 1) * PTS_CHUNK], in_=ot[:]
                )
                continue

            # ---- per-edge scalar weights ----
            xs = posx_sb[:, c0 : c0 + C_BLK]
            ys = posy_sb[:, c0 : c0 + C_BLK]

            win = None
            if a_exp > 0:
                xx = wpool.tile([128, C_BLK], f32, tag="xx")
                yy = wpool.tile([128, C_BLK], f32, tag="yy")
                d2 = wpool.tile([128, C_BLK], f32, tag="d2")
                nc.scalar.activation(xx[:], xs, Act.Square)
                nc.scalar.activation(yy[:], ys, Act.Square)
                nc.vector.tensor_tensor(out=d2[:], in0=xx[:], in1=yy[:], op=Alu.add)
                tw = wpool.tile([128, C_BLK], f32, tag="tw")
                nc.scalar.activation(tw[:], d2[:], Act.Relu, bias=1.0, scale=-inv_ws2)
                if a_exp == 1:
                    win = tw
                else:
                    t2 = wpool.tile([128, C_BLK], f32, tag="t2")
                    nc.scalar.activation(t2[:], tw[:], Act.Square)
                    if a_exp == 2:
                        win = t2
                    else:
                        win = wpool.tile([128, C_BLK], f32, tag="winp")
                        nc.vector.tensor_tensor(
                            out=win[:], in0=t2[:], in1=tw[:], op=Alu.mult
                        )
                        for _ in range(a_exp - 3):
                            nc.vector.tensor_tensor(
                                out=win[:], in0=win[:], in1=tw[:], op=Alu.mult
                            )

            gy = wpool.tile([128, C_BLK], f32, tag="gy")
            gx = wpool.tile([128, C_BLK], f32, tag="gx")
            nc.scalar.activation(gy[:], ys, Act.Relu, bias=c15[:], scale=s15)
            nc.scalar.activation(gx[:], xs, Act.Relu, bias=c15[:], scale=s15)
            nc.vector.tensor_scalar_min(gy[:], gy[:], 3.0)
            nc.vector.tensor_scalar_min(gx[:], gx[:], 3.0)

            # tent weights: w_j = relu(1 - |g - j|), j = 0..3
            def tents(gc, tag):
                td = wpool.tile([128, 4 * C_BLK], f32, tag=tag + "d", name=tag + "d")
                ta = wpool.tile([128, 4 * C_BLK], f32, tag=tag + "a", name=tag + "a")
                tww = wpool.tile([128, 4 * C_BLK], f32, tag=tag + "w", name=tag + "w")
                nc.vector.tensor_tensor(
                    out=td[:],
                    in0=gc[:].to_broadcast([128, C_BLK, 4]),
                    in1=bc(iota4[:], [(0, C_BLK), (1, 4)]),
                    op=Alu.subtract,
                )
                nc.scalar.activation(ta[:], td[:], Act.Abs)
                nc.scalar.activation(tww[:], ta[:], Act.Relu, bias=1.0, scale=-1.0)
                return tww

            wy = tents(gy, "ty")
            wx = tents(gx, "tx")
            if win is not None:
                wyw = wpool.tile([128, 4 * C_BLK], f32, tag="wyw")
                nc.vector.tensor_tensor(
                    out=wyw[:],
                    in0=wy[:],
                    in1=win[:].to_broadcast([128, C_BLK, 4]),
                    op=Alu.mult,
                )
            else:
                wyw = wy

            # ---- U block-diagonal writes: U[q, cb, jy, jx] for q's point grp ----
            for g4 in range(4):
                out_v = bc(
                    u[32 * g4 : 32 * g4 + 32, :],
                    [(64, C_BLK), (4, 4), (1, 4)],
                    extra_off=16 * g4,
                )
                in0 = bc(wyw[32 * g4 : 32 * g4 + 32, :], [(4, C_BLK), (1, 4), (0, 4)])
                in1 = bc(wx[32 * g4 : 32 * g4 + 32, :], [(4, C_BLK), (0, 4), (1, 4)])
                nc.vector.tensor_tensor(out=out_v, in0=in0, in1=in1, op=Alu.mult)

            if _dbg == "ubuild":
                ot = opool.tile([COUT, PTS_CHUNK], f32, tag="ot")
                nc.vector.tensor_copy(ot[:], u[0:COUT, 0:PTS_CHUNK])
                nc.sync.dma_start(
                    out=outT[:, ci * PTS_CHUNK : (ci + 1) * PTS_CHUNK], in_=ot[:]
                )
                continue

            # ---- stage 1: Mt[ch, 4pt*16g] per block ----
            mt = mpool.tile([CIN, C_BLK * 64], f16, tag="mt")
            for sub in range(8):
                ps = pspool.tile([64, 448], f32, tag="ps1")
                for b7 in range(7):
                    cb = sub * 7 + b7
                    nc.tensor.matmul(
                        ps[:, b7 * 64 : (b7 + 1) * 64],
                        lhsT=gt[:, cb * 2 * CIN : cb * 2 * CIN + CIN],
                        rhs=u[:, cb * 64 : (cb + 1) * 64],
                        start=True,
                        stop=True,
                    )
                nc.scalar.copy(out=mt[:, sub * 448 : (sub + 1) * 448], in_=ps[:])

            if _dbg == "mm1":
                ot = opool.tile([COUT, PTS_CHUNK], f32, tag="ot")
                nc.vector.tensor_copy(ot[:], mt[:, 0:PTS_CHUNK])
                nc.sync.dma_start(
                    out=outT[:, ci * PTS_CHUNK : (ci + 1) * PTS_CHUNK], in_=ot[:]
                )
                continue

            # ---- stage 2: out^T[oc, pts] = sum_g K_g^T @ Mt_g ----
            ps2 = pspool.tile([COUT, PTS_CHUNK], f32, tag="ps2")
            for g in range(G_BINS):
                nc.tensor.matmul(
                    ps2[:],
                    lhsT=kmat_sb[:, g * COUT : (g + 1) * COUT],
                    rhs=mt[:, g :: G_BINS],
                    start=(g == 0),
                    stop=(g == G_BINS - 1),
                )
            ot = opool.tile([COUT, PTS_CHUNK], f32, tag="ot")
            nc.vector.tensor_scalar(
                out=ot[:],
                in0=ps2[:],
                scalar1=1.0 / P_NBR,
                scalar2=bias_sb[:, 0:1],
                op0=Alu.mult,
                op1=Alu.add,
            )
            nc.sync.dma_start(
                out=outT[:, ci * PTS_CHUNK : (ci + 1) * PTS_CHUNK], in_=ot[:]
            )

    nc.compile()
    return nc


def kernel(features, receivers, relative_positions, window_support, a, kernel, bias):
    global LAST_EXEC_NS
    import os

    from concourse.bass_utils import run_bass_kernel_spmd

    features = np.ascontiguousarray(np.asarray(features, dtype=np.float32))
    recv = np.asarray(receivers).astype(np.int64)
    rel = np.asarray(relative_positions, dtype=np.float32)
    ws = float(np.asarray(window_support))
    a_exp = int(np.asarray(a))
    kern = np.asarray(kernel, dtype=np.float32)
    bias_np = np.asarray(bias, dtype=np.float32)

    key = (a_exp, round(ws, 9))
    if key not in _prog_cache:
        _prog_cache[key] = _build_nc(a_exp, 1.0 / (ws * ws), 1.5 / ws)
    nc = _prog_cache[key]

    # The neuron compile cache keys on the HLO shapes only, not the embedded
    # BIR — pin the cache dir to this kernel's source so edits never collide
    # with stale (possibly failed) cache entries.
    import hashlib

    try:
        with open(__file__, "rb") as f:
            src = f.read()
    except OSError:
        src = b""
    tag = hashlib.sha256(src + repr(key).encode()).hexdigest()[:16]
    os.environ["NEURON_COMPILE_CACHE_URL"] = f"/var/tmp/neuron-cc-{tag}"

    # ---- host-side layout prep (sharding) ----
    pad_n = NCORES * NPTS
    recv_pad = np.full((pad_n, P_NBR), HALF, dtype=np.int64)
    recv_pad[:N_FULL] = recv
    rel_pad = np.zeros((pad_n, P_NBR, 2), dtype=np.float32)
    rel_pad[:N_FULL] = rel

    # The gather ucode trims *trailing* negative int16 indices from each
    # 1024-index call, and each call ends on some point's last neighbor slot.
    # Reorder edges within each point (sum over neighbors is symmetric) so
    # slot 31 holds an index >= HALF whenever the point has one.
    last_neg = recv_pad[:, P_NBR - 1] < HALF
    has_pos = (recv_pad >= HALF).any(axis=1)
    fix = np.nonzero(last_neg & has_pos)[0]
    j = np.argmax(recv_pad[fix] >= HALF, axis=1)
    r31 = recv_pad[fix, P_NBR - 1].copy()
    p31 = rel_pad[fix, P_NBR - 1].copy()
    recv_pad[fix, P_NBR - 1] = recv_pad[fix, j]
    rel_pad[fix, P_NBR - 1] = rel_pad[fix, j]
    recv_pad[fix, j] = r31
    rel_pad[fix, j] = p31
    bad = np.nonzero(last_neg & ~has_pos)[0]
    # only call-final points matter; calls end at local point index 32k+31
    if bad.size:
        local = bad % NPTS
        assert not ((local % 32) == 31).any(), (
            "a gather call ends on a point whose 32 receiver indices are all "
            f"< {HALF}; trailing-trim would drop its edges"
        )

    kmat_np = np.ascontiguousarray(
        kern.reshape(G_BINS, CIN, COUT)
        .transpose(1, 0, 2)
        .reshape(CIN, G_BINS * COUT)
        .astype(np.float16)
    )
    bias_2d = np.ascontiguousarray(bias_np.reshape(COUT, 1))
    iota4_np = np.tile(np.arange(4, dtype=np.float32)[None, :], (128, 1))
    c15_np = np.full((128, 1), 1.5, dtype=np.float32)

    feat16 = np.zeros((N_FULL, 2 * CIN), dtype=np.float16)
    feat16[:, :CIN] = features.astype(np.float16)

    in_maps = []
    for c in range(NCORES):
        sl = slice(c * NPTS, (c + 1) * NPTS)
        # edge e = local_point*32 + nbr ; block b = e//128 ; slot q = e%128
        idx16 = (recv_pad[sl].reshape(-1) - HALF).astype(np.int16)
        # per call of 1024: idx i -> [i % 16, i // 16]; replicate over Q7 cores
        ncalls = NBLK // GCALL
        tbl16 = idx16.reshape(ncalls, PC_COLS, 16).transpose(2, 0, 1).reshape(
            16, ncalls * PC_COLS
        )
        idx_np = np.ascontiguousarray(np.tile(tbl16, (8, 1)))
        ry = np.ascontiguousarray(rel_pad[sl, :, 0].reshape(NBLK, 128).T)
        rx = np.ascontiguousarray(rel_pad[sl, :, 1].reshape(NBLK, 128).T)
        in_maps.append(
            {
                "feat": feat16,
                "idxs": idx_np,
                "posy": ry,
                "posx": rx,
                "kmat": kmat_np,
                "bias": bias_2d,
                "iot4": iota4_np,
                "c15d": c15_np,
            }
        )

    trace = bool(os.environ.get("KERNEL_TRACE"))
    res = run_bass_kernel_spmd(nc, in_maps, list(range(NCORES)), trace=trace)
    LAST_EXEC_NS = res.exec_time_ns

    out = np.concatenate(
        [res.results[c]["outT"].T for c in range(NCORES)], axis=0
    )
    return np.ascontiguousarray(out[:N_FULL])


# revision 16
# speedup vs baseline: 1.2796x; 1.2796x over previous
"""Trainium2 Bass kernel for the continuous-convolution (CConv) GNN layer.

Math (per output point n, P=32 neighbors, 4x4 bilinear kernel grid, 64->64 ch):
    gathered = features[receivers]                      # [N,P,64]
    win      = relu(1 - |r|^2/ws^2)^a                   # radial window
    gy,gx    = clip((r/ws + 1)*1.5, 0, 3)               # grid coords
    bilinear -> tent weights  w_j = relu(1 - |g - j|)   # j = 0..3 (exact)
    M[n,g]   = sum_p win * wy[jy] * wx[jx] * gathered   # g = 4*jy+jx
    out[n]   = (sum_g M[n,g] @ K[g]) / P + bias

Device mapping (8 NeuronCores, data-parallel over points):
  * 6272 points/core (50176 padded), edges blocked 128 = 4 points x 32 nbrs.
  * Gather: Q7 dma_gather from the HBM feature table. int16 indices cover
    all 50000 rows by pointing the source AP at row 25000 (signed offsets
    reach both halves); a trailing pad block per call keeps the last index
    non-negative (the ucode trims trailing negatives).
  * Stage 1 (PE): per 128-edge block  Mt = G^T @ U : lhsT = gathered G
    [128e, 64ch], rhs = U [128e, 64] block-diagonal bilinear weights
    (4 points x 16 bins) -> psum [64ch, 4pt*16g].
  * Stage 2 (PE): out^T[oc, pts] += K_g^T[oc,ch] @ Mt_g[ch, pts], 16 bins
    accumulated in PSUM; then *1/P + bias on DVE; out stored transposed,
    host transposes back (pure layout).
"""

import sys

sys.path.insert(0, "/opt/trn_rl_repo")

import dataclasses
from contextlib import ExitStack

import numpy as np

N_FULL = 50000
HALF = 25000             # gather base row: int16 idx = r - HALF
P_NBR = 32
CIN = 64
COUT = 64
G_BINS = 16
NCORES = 8
NPTS = 6272              # padded points per core; 8*6272 = 50176 >= 50000
NBLK = NPTS // 4         # 1568 blocks of 128 edges
C_BLK = 56               # real blocks per pipeline chunk
NCHUNK = NBLK // C_BLK   # 28
GCALL = 8                # blocks per dma_gather call (1024-descriptor ring cap)
NCALL_C = C_BLK // GCALL  # gather calls per chunk (7)
PERCALL = GCALL * 128    # indices per gather call (1024)
PC_COLS = PERCALL // 16  # idx columns per call (64)
PTS_CHUNK = C_BLK * 4    # 224 points produced per chunk

_prog_cache = {}
LAST_EXEC_NS = None


def _build_nc(a_exp, inv_ws2, s15):
    import concourse.bacc as bacc
    import concourse.bass as bass
    import concourse.mybir as mybir
    from concourse.tile import TileContext
    from concourse.vector_clock import ScopedClock, VectorClock

    f32 = mybir.dt.float32
    f16 = mybir.dt.float16
    i16 = mybir.dt.int16
    Alu = mybir.AluOpType
    Act = mybir.ActivationFunctionType

    class TC(TileContext):
        # The stock final drain packs every outstanding semaphore wait onto a
        # single Drain instruction; walrus here accepts at most one sync-wait
        # per CTRL instruction. Emit one drain per outstanding sem lane.
        def _drain_and_barrier(self, tick_clock, wait_clock):
            nc = self.nc
            ticks = eval(repr(tick_clock.global_clock).replace("VectorClock", ""))
            nz = [i for i, t in enumerate(ticks) if t > 0]
            if not nz:
                nc.sync.drain()
            for i in nz:
                part = [ticks[j] if j == i else 0 for j in range(len(ticks))]
                d = nc.sync.drain()
                wait_clock.add_sem_waits(d.ins, ScopedClock({None: VectorClock(part)}))
            nc.all_engine_barrier()
            popped = nc._tile_sem_poison_stack.pop()
            assert popped is self._sem_poison
            nc.clear_and_free_semaphores(list(self.sems.allocated().values()))
            nc.all_engine_barrier()

    def bc(view, dims, extra_off=0):
        # hand-built access pattern: keep partition dim, replace free dims
        return dataclasses.replace(
            view,
            ap=[view.ap[0]] + [list(d) for d in dims],
            offset=view.offset + extra_off,
        )

    nc = bacc.Bacc(
        "TRN2", target_bir_lowering=False, debug=False, num_swdge_queues=4
    )
    feat = nc.declare_dram_parameter("feat", [N_FULL, 2 * CIN], f16, isOutput=False)
    idxs = nc.declare_dram_parameter("idxs", [128, NBLK * 8], i16, isOutput=False)
    posy = nc.declare_dram_parameter("posy", [128, NBLK], f32, isOutput=False)
    posx = nc.declare_dram_parameter("posx", [128, NBLK], f32, isOutput=False)
    kmat = nc.declare_dram_parameter("kmat", [CIN, G_BINS * COUT], f16, isOutput=False)
    bias = nc.declare_dram_parameter("bias", [COUT, 1], f32, isOutput=False)
    iot4 = nc.declare_dram_parameter("iot4", [128, 4], f32, isOutput=False)
    c15d = nc.declare_dram_parameter("c15d", [128, 1], f32, isOutput=False)
    c3d = nc.declare_dram_parameter("c3d", [128, 1], f32, isOutput=False)
    outT = nc.declare_dram_parameter("outT", [COUT, NPTS], f32, isOutput=True)

    with TC(nc) as tc, ExitStack() as ctx:
        const = ctx.enter_context(tc.tile_pool(name="const", bufs=1))
        gpool = ctx.enter_context(tc.tile_pool(name="g", bufs=3))
        wpool = ctx.enter_context(tc.tile_pool(name="w", bufs=3))
        mpool = ctx.enter_context(tc.tile_pool(name="mt", bufs=3))
        opool = ctx.enter_context(tc.tile_pool(name="ot", bufs=3))
        pspool = ctx.enter_context(tc.tile_pool(name="ps", bufs=3, space="PSUM"))

        idx_sb = const.tile([128, NBLK * 8], i16)
        posy_sb = const.tile([128, NBLK], f32)
        posx_sb = const.tile([128, NBLK], f32)
        kmat_sb = const.tile([CIN, G_BINS * COUT], f16)
        bias_sb = const.tile([COUT, 1], f32)
        iota4 = const.tile([128, 4], f32)
        c15 = const.tile([128, 1], f32)
        c3 = const.tile([128, 1], f32)
        # U tiles keep their block-diagonal zero regions across chunks
        u_bufs = [
            const.tile([128, C_BLK * 64], f16, tag="u0", name="u0"),
            const.tile([128, C_BLK * 64], f16, tag="u1", name="u1"),
            const.tile([128, C_BLK * 64], f16, tag="u2", name="u2"),
        ]

        nc.sync.dma_start(out=idx_sb[:], in_=idxs[:])
        nc.sync.dma_start(out=posy_sb[:], in_=posy[:])
        nc.sync.dma_start(out=posx_sb[:], in_=posx[:])
        nc.sync.dma_start(out=kmat_sb[:], in_=kmat[:])
        nc.sync.dma_start(out=bias_sb[:], in_=bias[:])
        nc.sync.dma_start(out=iota4[:], in_=iot4[:])
        nc.sync.dma_start(out=c15[:], in_=c15d[:])
        nc.sync.dma_start(out=c3[:], in_=c3d[:])
        nc.vector.memset(u_bufs[0][:], 0.0)
        nc.vector.memset(u_bufs[1][:], 0.0)
        nc.vector.memset(u_bufs[2][:], 0.0)

        import os as _os

        _nchunk = int(_os.environ.get("KERNEL_NCHUNK", NCHUNK))
        _dbg = _os.environ.get("KERNEL_DEBUG", "full")
        for ci in range(_nchunk):
            c0 = ci * C_BLK
            u = u_bufs[ci % 3]

            # ---- gather: 56 blocks of feature rows, 8 blocks per call ----
            # (fp16 rows padded to 128ch = 256B elements; desc-gen spread
            #  over the 4 SWDGE queues = 4 Q7 core pairs)
            gt = gpool.tile([128, C_BLK * 2 * CIN], f16, tag="gt")
            for sc in range(NCALL_C):
                gv = dataclasses.replace(
                    gt[:],
                    ap=[gt[:].ap[0], [2 * CIN, GCALL], [1, 2 * CIN]],
                    offset=gt[:].offset + sc * GCALL * 2 * CIN,
                )
                col0 = (ci * NCALL_C + sc) * PC_COLS
                nc.gpsimd.dma_gather(
                    out_ap=gv,
                    in_ap=feat[HALF:, :],
                    idxs_ap=idx_sb[:, col0 : col0 + PC_COLS],
                    num_idxs=PERCALL,
                    num_idxs_reg=PERCALL,
                    elem_size=2 * CIN,
                    elem_step=2 * CIN,
                    queue_num=(ci * NCALL_C + sc) % 4,
                )

            if _dbg == "gather":
                ot = opool.tile([COUT, PTS_CHUNK], f32, tag="ot")
                nc.vector.tensor_copy(ot[:], gt[0:COUT, 0:PTS_CHUNK])
                nc.sync.dma_start(
                    out=outT[:, ci * PTS_CHUNK : (ci + 1) * PTS_CHUNK], in_=ot[:]
                )
                continue

            # ---- per-edge scalar weights ----
            xs = posx_sb[:, c0 : c0 + C_BLK]
            ys = posy_sb[:, c0 : c0 + C_BLK]

            win = None
            if a_exp > 0:
                xx = wpool.tile([128, C_BLK], f32, tag="xx")
                yy = wpool.tile([128, C_BLK], f32, tag="yy")
                d2 = wpool.tile([128, C_BLK], f32, tag="d2")
                nc.scalar.activation(xx[:], xs, Act.Square)
                nc.scalar.activation(yy[:], ys, Act.Square)
                nc.vector.tensor_tensor(out=d2[:], in0=xx[:], in1=yy[:], op=Alu.add)
                tw = wpool.tile([128, C_BLK], f32, tag="tw")
                nc.scalar.activation(tw[:], d2[:], Act.Relu, bias=1.0, scale=-inv_ws2)
                if a_exp == 1:
                    win = tw
                else:
                    t2 = wpool.tile([128, C_BLK], f32, tag="t2")
                    nc.scalar.activation(t2[:], tw[:], Act.Square)
                    if a_exp == 2:
                        win = t2
                    else:
                        win = wpool.tile([128, C_BLK], f32, tag="winp")
                        nc.vector.tensor_tensor(
                            out=win[:], in0=t2[:], in1=tw[:], op=Alu.mult
                        )
                        for _ in range(a_exp - 3):
                            nc.vector.tensor_tensor(
                                out=win[:], in0=win[:], in1=tw[:], op=Alu.mult
                            )

            # rc = Relu(3 - Relu(1.5*y + 1.5))  =>  gy_clipped = 3 - rc
            gy = wpool.tile([128, C_BLK], f32, tag="gy")
            gx = wpool.tile([128, C_BLK], f32, tag="gx")
            nc.scalar.activation(gy[:], ys, Act.Relu, bias=c15[:], scale=s15)
            nc.scalar.activation(gx[:], xs, Act.Relu, bias=c15[:], scale=s15)
            nc.scalar.activation(gy[:], gy[:], Act.Relu, bias=c3[:], scale=-1.0)
            nc.scalar.activation(gx[:], gx[:], Act.Relu, bias=c3[:], scale=-1.0)

            # tent weights: w_j = relu(1 - |g - j|) with g = 3 - rc:
            # g - j = (3 - j) - rc, so subtract rc from the reversed iota.
            def tents(rc, tag):
                td = wpool.tile([128, 4 * C_BLK], f32, tag=tag + "d", name=tag + "d")
                ta = wpool.tile([128, 4 * C_BLK], f32, tag=tag + "a", name=tag + "a")
                tww = wpool.tile([128, 4 * C_BLK], f32, tag=tag + "w", name=tag + "w")
                nc.vector.tensor_tensor(
                    out=td[:],
                    in0=bc(iota4[:], [(0, C_BLK), (1, 4)]),
                    in1=rc[:].to_broadcast([128, C_BLK, 4]),
                    op=Alu.subtract,
                )
                nc.scalar.activation(ta[:], td[:], Act.Abs)
                nc.scalar.activation(tww[:], ta[:], Act.Relu, bias=1.0, scale=-1.0)
                return tww

            wy = tents(gy, "ty")
            wx = tents(gx, "tx")
            if win is not None:
                wyw = wpool.tile([128, 4 * C_BLK], f32, tag="wyw")
                nc.vector.tensor_tensor(
                    out=wyw[:],
                    in0=wy[:],
                    in1=win[:].to_broadcast([128, C_BLK, 4]),
                    op=Alu.mult,
                )
            else:
                wyw = wy

            # ---- U block-diagonal writes: U[q, cb, jy, jx] for q's point grp ----
            for g4 in range(4):
                out_v = bc(
                    u[32 * g4 : 32 * g4 + 32, :],
                    [(64, C_BLK), (4, 4), (1, 4)],
                    extra_off=16 * g4,
                )
                in0 = bc(wyw[32 * g4 : 32 * g4 + 32, :], [(4, C_BLK), (1, 4), (0, 4)])
                in1 = bc(wx[32 * g4 : 32 * g4 + 32, :], [(4, C_BLK), (0, 4), (1, 4)])
                nc.vector.tensor_tensor(out=out_v, in0=in0, in1=in1, op=Alu.mult)

            if _dbg == "ubuild":
                ot = opool.tile([COUT, PTS_CHUNK], f32, tag="ot")
                nc.vector.tensor_copy(ot[:], u[0:COUT, 0:PTS_CHUNK])
                nc.sync.dma_start(
                    out=outT[:, ci * PTS_CHUNK : (ci + 1) * PTS_CHUNK], in_=ot[:]
                )
                continue

            # ---- stage 1: Mt[ch, 4pt*16g] per block ----
            mt = mpool.tile([CIN, C_BLK * 64], f16, tag="mt")
            for sub in range(8):
                ps = pspool.tile([64, 448], f32, tag="ps1")
                for b7 in range(7):
                    cb = sub * 7 + b7
                    nc.tensor.matmul(
                        ps[:, b7 * 64 : (b7 + 1) * 64],
                        lhsT=gt[:, cb * 2 * CIN : cb * 2 * CIN + CIN],
                        rhs=u[:, cb * 64 : (cb + 1) * 64],
                        start=True,
                        stop=True,
                    )
                nc.scalar.copy(out=mt[:, sub * 448 : (sub + 1) * 448], in_=ps[:])

            if _dbg == "mm1":
                ot = opool.tile([COUT, PTS_CHUNK], f32, tag="ot")
                nc.vector.tensor_copy(ot[:], mt[:, 0:PTS_CHUNK])
                nc.sync.dma_start(
                    out=outT[:, ci * PTS_CHUNK : (ci + 1) * PTS_CHUNK], in_=ot[:]
                )
                continue

            # ---- stage 2: out^T[oc, pts] = sum_g K_g^T @ Mt_g ----
            ps2 = pspool.tile([COUT, PTS_CHUNK], f32, tag="ps2")
            for g in range(G_BINS):
                nc.tensor.matmul(
                    ps2[:],
                    lhsT=kmat_sb[:, g * COUT : (g + 1) * COUT],
                    rhs=mt[:, g :: G_BINS],
                    start=(g == 0),
                    stop=(g == G_BINS - 1),
                )
            ot = opool.tile([COUT, PTS_CHUNK], f32, tag="ot")
            nc.scalar.activation(
                ot[:], ps2[:], Act.Identity, bias=bias_sb[:, 0:1], scale=1.0 / P_NBR
            )
            nc.sync.dma_start(
                out=outT[:, ci * PTS_CHUNK : (ci + 1) * PTS_CHUNK], in_=ot[:]
            )

    nc.compile()
    return nc


def kernel(features, receivers, relative_positions, window_support, a, kernel, bias):
    global LAST_EXEC_NS
    import os

    from concourse.bass_utils import run_bass_kernel_spmd

    features = np.ascontiguousarray(np.asarray(features, dtype=np.float32))
    recv = np.asarray(receivers).astype(np.int64)
    rel = np.asarray(relative_positions, dtype=np.float32)
    ws = float(np.asarray(window_support))
    a_exp = int(np.asarray(a))
    kern = np.asarray(kernel, dtype=np.float32)
    bias_np = np.asarray(bias, dtype=np.float32)

    key = (a_exp, round(ws, 9))
    if key not in _prog_cache:
        _prog_cache[key] = _build_nc(a_exp, 1.0 / (ws * ws), 1.5 / ws)
    nc = _prog_cache[key]

    # The neuron compile cache keys on the HLO shapes only, not the embedded
    # BIR — pin the cache dir to this kernel's source so edits never collide
    # with stale (possibly failed) cache entries.
    import hashlib

    try:
        with open(__file__, "rb") as f:
            src = f.read()
    except OSError:
        src = b""
    tag = hashlib.sha256(src + repr(key).encode()).hexdigest()[:16]
    os.environ["NEURON_COMPILE_CACHE_URL"] = f"/var/tmp/neuron-cc-{tag}"

    # ---- host-side layout prep (sharding) ----
    pad_n = NCORES * NPTS
    recv_pad = np.full((pad_n, P_NBR), HALF, dtype=np.int64)
    recv_pad[:N_FULL] = recv
    rel_pad = np.zeros((pad_n, P_NBR, 2), dtype=np.float32)
    rel_pad[:N_FULL] = rel

    # The gather ucode trims *trailing* negative int16 indices from each
    # 1024-index call, and each call ends on some point's last neighbor slot.
    # Reorder edges within each point (sum over neighbors is symmetric) so
    # slot 31 holds an index >= HALF whenever the point has one.
    last_neg = recv_pad[:, P_NBR - 1] < HALF
    has_pos = (recv_pad >= HALF).any(axis=1)
    fix = np.nonzero(last_neg & has_pos)[0]
    j = np.argmax(recv_pad[fix] >= HALF, axis=1)
    r31 = recv_pad[fix, P_NBR - 1].copy()
    p31 = rel_pad[fix, P_NBR - 1].copy()
    recv_pad[fix, P_NBR - 1] = recv_pad[fix, j]
    rel_pad[fix, P_NBR - 1] = rel_pad[fix, j]
    recv_pad[fix, j] = r31
    rel_pad[fix, j] = p31
    bad = np.nonzero(last_neg & ~has_pos)[0]
    # only call-final points matter; calls end at local point index 32k+31
    if bad.size:
        local = bad % NPTS
        assert not ((local % 32) == 31).any(), (
            "a gather call ends on a point whose 32 receiver indices are all "
            f"< {HALF}; trailing-trim would drop its edges"
        )

    kmat_np = np.ascontiguousarray(
        kern.reshape(G_BINS, CIN, COUT)
        .transpose(1, 0, 2)
        .reshape(CIN, G_BINS * COUT)
        .astype(np.float16)
    )
    bias_2d = np.ascontiguousarray(bias_np.reshape(COUT, 1))
    iota4_np = np.tile(
        np.array([3.0, 2.0, 1.0, 0.0], dtype=np.float32)[None, :], (128, 1)
    )
    c15_np = np.full((128, 1), 1.5, dtype=np.float32)
    c3_np = np.full((128, 1), 3.0, dtype=np.float32)

    feat16 = np.zeros((N_FULL, 2 * CIN), dtype=np.float16)
    feat16[:, :CIN] = features.astype(np.float16)

    in_maps = []
    for c in range(NCORES):
        sl = slice(c * NPTS, (c + 1) * NPTS)
        # edge e = local_point*32 + nbr ; block b = e//128 ; slot q = e%128
        idx16 = (recv_pad[sl].reshape(-1) - HALF).astype(np.int16)
        # per call of 1024: idx i -> [i % 16, i // 16]; replicate over Q7 cores
        ncalls = NBLK // GCALL
        tbl16 = idx16.reshape(ncalls, PC_COLS, 16).transpose(2, 0, 1).reshape(
            16, ncalls * PC_COLS
        )
        idx_np = np.ascontiguousarray(np.tile(tbl16, (8, 1)))
        ry = np.ascontiguousarray(rel_pad[sl, :, 0].reshape(NBLK, 128).T)
        rx = np.ascontiguousarray(rel_pad[sl, :, 1].reshape(NBLK, 128).T)
        in_maps.append(
            {
                "feat": feat16,
                "idxs": idx_np,
                "posy": ry,
                "posx": rx,
                "kmat": kmat_np,
                "bias": bias_2d,
                "iot4": iota4_np,
                "c15d": c15_np,
                "c3d": c3_np,
            }
        )

    trace = bool(os.environ.get("KERNEL_TRACE"))
    res = run_bass_kernel_spmd(nc, in_maps, list(range(NCORES)), trace=trace)
    LAST_EXEC_NS = res.exec_time_ns

    out = np.concatenate(
        [res.results[c]["outT"].T for c in range(NCORES)], axis=0
    )
    return np.ascontiguousarray(out[:N_FULL])
